# revision 13
# baseline (speedup 1.0000x reference)
"""Trainium2 Bass kernel for CustomAttentionWithPE (bf16 pipeline).

Reference computation (B=2, S=2048, H=16, Dh=64, D=1024):
    qkv = hs @ W_qkv + b_qkv ; split to q,k,v per head
    q,k = RoPE(q), RoPE(k)
    out = softmax(q k^T / 8) v   (no mask)
    return concat_heads(out) @ W_o + b_o

Sharding: 8 cores -> (batch b = core//4, head-quad g = core%4, heads 4g..4g+3).
Each core computes partial = attn(heads of g, batch b) @ W_o[rows of g]
for its batch; host sums the 4 partials per batch and adds the bias terms
(b_o and b_v @ W_o; softmax rows sum to 1 so the V bias contributes exactly
b_v @ W_o per token).

v2 design (vs fp32 baseline at ~800us):
  - All matmuls in bf16 (1 cyc/row vs fp32's 4); inputs converted to bf16
    on host, halving DMA. PSUM accumulation stays fp32.
  - Softmax exp on ScalarE is the new bottleneck (~147us: 16.8M elems +
    352cyc/instr overhead). Program order starts attention as early as
    possible (K-hp0 + Q-qt0 projections first) and interleaves V/remaining
    QKV/WO under the exp-bound attention phases so ACT is never starved.
  - Score PSUM tiles are [128, 2(heads), 512] so ONE activation instruction
    exps both heads of a pair per k-tile (fewer fixed overheads).
  - 1/Z via reciprocal_approx_fast (f32, ~5x faster than DVE reciprocal)
    directly off the PSUM Z row; broadcast across partitions by a rank-1
    bf16 matmul into the pv bank's upper 64 partitions.
  - RoPE via host-precomputed sign-folded sin (rows 0-31 negated) so the
    rotate-half is 4 shifted tensor_tensor muls + mul + add, all bf16.
"""

import math
from contextlib import ExitStack

import numpy as np
import ml_dtypes

import concourse.bass as bass
import concourse.mybir as mybir
import concourse.tile as tile
from concourse.bass_utils import run_bass_kernel_spmd

F32 = mybir.dt.float32
BF16 = mybir.dt.bfloat16
AF = mybir.ActivationFunctionType
NPBF16 = ml_dtypes.bfloat16

B, S, D = 2, 2048, 1024
NH, HD = 16, 64
ROPE_BASE = 10000.0
N_CORES = 8
HPC = 4  # heads per core
DLOC = HPC * HD  # 256 local head dims per core


def _split_sync_waits(nc, maxw=1):
    """This container's walrus rejects >1-2 SyncWaits per instruction
    ("Too many sync wait commands"). Move excess waits onto NoOps."""
    for f in nc.m.functions:
        for blk in f.blocks:
            new_instructions = []
            for ins in blk.instructions:
                si = getattr(ins, "sync_info", None)
                if si is not None and si.on_wait and len(si.on_wait) > maxw:
                    waits = list(si.on_wait)
                    extra, keep = waits[:-maxw], waits[-maxw:]
                    si.on_wait = keep
                    for i in range(0, len(extra), maxw):
                        nop = mybir.InstNoOp(
                            name=nc.get_next_instruction_name(),
                            engine=ins.engine,
                            sync_info=mybir.SyncInfo(
                                on_wait=extra[i : i + maxw], on_update=[]
                            ),
                        )
                        nc.register_instruction(nop, overwrite=True)
                        new_instructions.append(nop)
                new_instructions.append(ins)
            blk.instructions[:] = new_instructions


def build_attention_nc(seq=S, add_qk_bias=False, order="interleaved"):
    """One SPMD program; per-core data differs only through inputs."""
    nc = bass.Bass()
    NT = seq // 512  # 512-token stripes
    KT = seq // 128  # k tiles
    NCH = D // 128  # contraction chunks over d_model

    xT = nc.dram_tensor("xT", [D, seq], BF16, kind="ExternalInput")
    wq = nc.dram_tensor("wq", [D, DLOC], BF16, kind="ExternalInput")
    wk = nc.dram_tensor("wk", [D, DLOC], BF16, kind="ExternalInput")
    wv = nc.dram_tensor("wv", [D, DLOC], BF16, kind="ExternalInput")
    wo = nc.dram_tensor("wo", [DLOC, D], BF16, kind="ExternalInput")
    cosT = nc.dram_tensor("cosT", [HD, seq], BF16, kind="ExternalInput")
    sinT = nc.dram_tensor("sinT", [HD, seq], BF16, kind="ExternalInput")
    bqk = nc.dram_tensor("bqk", [2, DLOC], F32, kind="ExternalInput")
    out = nc.dram_tensor("out", [seq, D], BF16, kind="ExternalOutput")

    with tile.TileContext(nc) as tc, ExitStack() as ctx:
        consts = ctx.enter_context(tc.tile_pool(name="consts", bufs=1))
        # warm the exp table set before any real activation
        warm = consts.tile([1, 2], F32)
        nc.vector.memset(warm, 0.0)
        nc.scalar.activation(warm, warm, AF.Exp, scale=1.0)

        # DMA dispatch order = need order: wk + rope tables + x stripe 0
        # feed the very first K projection; wq next (Q stripe 0); the rest
        # stream in behind them.
        wk_sb = consts.tile([128, NCH, DLOC], BF16)
        nc.sync.dma_start(out=wk_sb, in_=wk.rearrange("(c p) m -> p c m", p=128))
        cs_sb = consts.tile([128, seq], BF16)
        nc.sync.dma_start(out=cs_sb[0:HD, :], in_=cosT[:])
        nc.sync.dma_start(out=cs_sb[HD:128, :], in_=cosT[:])
        sn_sb = consts.tile([128, seq], BF16)
        nc.sync.dma_start(out=sn_sb[0:HD, :], in_=sinT[:])
        nc.sync.dma_start(out=sn_sb[HD:128, :], in_=sinT[:])
        x_sb = consts.tile([128, NCH, seq], BF16)

        def emit_x_stripe(nt):
            for c in range(NCH):
                nc.sync.dma_start(
                    out=x_sb[:, c, nt * 512 : nt * 512 + 512],
                    in_=xT[c * 128 : (c + 1) * 128, nt * 512 : nt * 512 + 512],
                )

        emit_x_stripe(0)
        wq_sb = consts.tile([128, NCH, DLOC], BF16)
        nc.sync.dma_start(out=wq_sb, in_=wq.rearrange("(c p) m -> p c m", p=128))
        for nt in range(1, NT):
            emit_x_stripe(nt)
        wv_sb = consts.tile([128, NCH, DLOC], BF16)
        nc.sync.dma_start(out=wv_sb, in_=wv.rearrange("(c p) m -> p c m", p=128))
        wo_sb = consts.tile([128, 2, D], BF16)
        nc.sync.dma_start(out=wo_sb, in_=wo.rearrange("(c p) m -> p c m", p=128))
        ones_sb = consts.tile([128, HD], BF16)
        nc.vector.memset(ones_sb, 1.0)
        if add_qk_bias:
            bqk_sb = consts.tile([128, 2, 2], F32)
            nc.sync.dma_start(
                out=bqk_sb, in_=bqk.rearrange("b (h p) -> p b h", p=128)
            )

        # long-lived activation tensors
        acts = ctx.enter_context(tc.tile_pool(name="acts", bufs=1))
        qtr = acts.tile([128, 2, seq], BF16)  # RoPE'd Q^T, head pairs
        ktr = acts.tile([128, 2, seq], BF16)
        v_sb = acts.tile([128, KT, HPC, HD + 1], BF16)  # V natural + ones col
        att = acts.tile([128, 2, seq], BF16)  # normalized attn out ^T
        nc.vector.memset(v_sb[:, :, :, HD : HD + 1], 1.0)

        # pools
        ps = ctx.enter_context(tc.tile_pool(name="ps", bufs=1, space="PSUM"))
        rope_pool = ctx.enter_context(tc.tile_pool(name="rope", bufs=3))
        pt_pool = ctx.enter_context(tc.tile_pool(name="ptp", bufs=6))
        npool = ctx.enter_context(tc.tile_pool(name="norm", bufs=2))
        opool = ctx.enter_context(tc.tile_pool(name="ostage", bufs=2))

        def emit_qk(dst, w_sb, hp, nt, name):
            """Project one [128 dims, 512 tok] stripe of q^T or k^T and RoPE it."""
            cs = slice(nt * 512, nt * 512 + 512)
            pst = ps.tile([128, 512], F32, tag="qkv", bufs=2, name=f"ps_{name}")
            for c in range(NCH):
                nc.tensor.matmul(
                    pst,
                    w_sb[:, c, hp * 128 : hp * 128 + 128],
                    x_sb[:, c, cs],
                    start=(c == 0),
                    stop=(c == NCH - 1),
                )
            raw = rope_pool.tile([128, 512], BF16, tag="raw", name=f"raw_{name}")
            nc.vector.tensor_copy(raw, pst)
            if add_qk_bias:
                bi = 0 if dst is qtr else 1
                nc.vector.tensor_scalar_add(raw, raw, bqk_sb[:, bi, hp : hp + 1])
            rot = rope_pool.tile([128, 512], BF16, tag="rot", name=f"rot_{name}")
            for b in (0, 64):
                nc.vector.tensor_copy(rot[b : b + 32, :], raw[b + 32 : b + 64, :])
                nc.vector.tensor_copy(rot[b + 32 : b + 64, :], raw[b : b + 32, :])
            nc.vector.tensor_mul(rot, rot, sn_sb[:, cs])
            tmp = rope_pool.tile([128, 512], BF16, tag="tmp", name=f"tmp_{name}")
            nc.vector.tensor_mul(tmp, raw, cs_sb[:, cs])
            nc.vector.tensor_add(dst[:, hp, cs], tmp, rot)

        def emit_v_chunk(kt_idx):
            """V natural [128 tokens, vcol] for one k-tile, all 4 heads."""
            vp = ps.tile([128, HPC, HD], F32, tag="qkv", bufs=2, name="ps_v")
            for c in range(NCH):
                nc.tensor.matmul(
                    vp,
                    x_sb[:, c, kt_idx * 128 : kt_idx * 128 + 128],
                    wv_sb[:, c, :],
                    start=(c == 0),
                    stop=(c == NCH - 1),
                )
            nc.vector.tensor_copy(v_sb[:, kt_idx, :, 0:HD], vp)

        def emit_v(nt):
            for tt in range(4):
                emit_v_chunk(nt * 4 + tt)

        def emit_attn(qt, hp, pre_kt=None, pre_pv=None):
            """scores -> exp -> PV -> normalize(A) for one (q-stripe, pair).

            Tile trace order IS program order, so hooks place prerequisite /
            filler work at exact trace positions:
              pre_kt[kt]: before that kt's score matmuls
              pre_pv[kt]: between the exp and that kt's PV matmuls (used for
                          the V chunk feeding exactly that PV)
            Returns a finish closure (norm part B: 1/Z broadcast + multiply)
            that the caller emits later — off the PE-critical path, after the
            reciprocal has had time to complete."""
            qs = slice(qt * 512, qt * 512 + 512)
            pv = [
                ps.tile([128, 512], F32, tag="pv", bufs=2, name="pv0"),
                ps.tile([128, 512], F32, tag="pv", bufs=2, name="pv1"),
            ]
            for kt in range(KT):
                if pre_kt and kt in pre_kt:
                    pre_kt[kt]()
                sc = ps.tile([128, 2, 512], F32, tag="sc", bufs=2, name="sc")
                for h in range(2):
                    hb = h * 64
                    nc.tensor.matmul(
                        sc[:, h, :],
                        ktr[hb : hb + 64, hp, kt * 128 : kt * 128 + 128],
                        qtr[hb : hb + 64, hp, qs],
                        start=True,
                        stop=True,
                    )
                pt = pt_pool.tile([128, 2, 512], BF16, tag="pt", bufs=6, name="pt")
                nc.scalar.activation(pt, sc, AF.Exp, scale=0.125)
                if pre_pv and kt in pre_pv:
                    pre_pv[kt]()
                for h in range(2):
                    nc.tensor.matmul(
                        pv[h][0 : HD + 1, :],
                        v_sb[:, kt, hp * 2 + h, :],
                        pt[:, h, :],
                        start=(kt == 0),
                        stop=(kt == KT - 1),
                        skip_group_check=True,
                    )
            # norm part A: stage O'/Z off PSUM (frees the pv slots) and start
            # the reciprocal. tensor_tensor needs same start partitions, so
            # partition shifts are done with copies (exempt from the rule).
            o_sb = npool.tile([128, 512], BF16, tag="osb", name="osb")
            zstage = npool.tile([65, 512], F32, tag="zstage", name="zstage")
            zrecf = npool.tile([65, 512], F32, tag="zrecf", name="zrecf")
            zrec = npool.tile([65, 512], BF16, tag="zrec", name="zrec")
            for h in range(2):
                hb = h * 64
                nc.vector.tensor_copy(o_sb[hb : hb + 64, :], pv[h][0:HD, :])
                nc.vector.tensor_copy(zstage[hb : hb + 1, :], pv[h][HD : HD + 1, :])
            # one partition-parallel reciprocal covers both heads' Z rows
            # (rows 1..63 are don't-care lanes; same wall time as [1,512])
            nc.vector.reciprocal(zrecf[0:HD + 1, :], zstage[0:HD + 1, :])
            for h in range(2):
                hb = h * 64
                nc.vector.tensor_copy(zrec[hb : hb + 1, :], zrecf[hb : hb + 1, :])

            def finish():
                # norm part B: rank-1 broadcast of 1/Z + un-normalized O' mul
                zb = ps.tile([128, 512], F32, tag="qkv", bufs=2, name="zb")
                for h in range(2):
                    hb = h * 64
                    nc.tensor.matmul(
                        zb[hb : hb + 64, :],
                        ones_sb[hb : hb + 1, 0:HD],
                        zrec[hb : hb + 1, :],
                        start=True,
                        stop=True,
                        skip_group_check=True,
                    )
                    nc.vector.tensor_mul(
                        att[hb : hb + 64, hp, qs],
                        o_sb[hb : hb + 64, :],
                        zb[hb : hb + 64, :],
                    )

            return finish

        def emit_wo(qt):
            """Output projection + store for one 512-token stripe."""
            for tt in range(4):
                tok = qt * 512 + tt * 128
                for nh in range(2):
                    pw = ps.tile([128, 512], F32, tag="qkv", bufs=2, name="ps_wo")
                    for hp in range(2):
                        nc.tensor.matmul(
                            pw,
                            att[:, hp, tok : tok + 128],
                            wo_sb[:, hp, nh * 512 : nh * 512 + 512],
                            start=(hp == 0),
                            stop=(hp == 1),
                        )
                    o_out = opool.tile([128, 512], BF16, tag="oo", name="oo")
                    nc.vector.tensor_copy(o_out, pw)
                    nc.sync.dma_start(
                        out=out[tok : tok + 128, nh * 512 : nh * 512 + 512],
                        in_=o_out,
                    )

        if order == "serial":
            for hp in range(2):
                for nt in range(NT):
                    emit_qk(ktr, wk_sb, hp, nt, f"k{hp}_{nt}")
                    emit_qk(qtr, wq_sb, hp, nt, f"q{hp}_{nt}")
            for nt in range(NT):
                emit_v(nt)
            for hp in range(2):
                for qt in range(NT):
                    emit_attn(qt, hp)()
            for qt in range(NT):
                emit_wo(qt)
        else:
            # Trace order = program order = scheduler priority. Attention
            # feeds the bottleneck ScalarE exp stream, so it leads; K/Q
            # stripe projections, V chunks, deferred norm finishes and WO are
            # threaded into exact positions where their outputs are first
            # needed (or as PE filler).
            emit_qk(ktr, wk_sb, 0, 0, "k0_0")
            emit_qk(qtr, wq_sb, 0, 0, "q0_0")
            fin = emit_attn(
                0, 0,
                pre_kt={2: lambda: emit_qk(ktr, wk_sb, 0, 1, "k0_1"),
                        6: lambda: emit_qk(ktr, wk_sb, 0, 2, "k0_2"),
                        10: lambda: emit_qk(ktr, wk_sb, 0, 3, "k0_3"),
                        13: lambda: emit_qk(qtr, wq_sb, 0, 1, "q0_1")},
                pre_pv={kt: (lambda k=kt: emit_v_chunk(k)) for kt in range(KT)},
            )
            fin = [fin, emit_attn(
                1, 0,
                pre_kt={3: fin,
                        6: lambda: emit_qk(qtr, wq_sb, 0, 2, "q0_2"),
                        11: lambda: emit_qk(qtr, wq_sb, 0, 3, "q0_3")},
            )][1]
            fin = [fin, emit_attn(
                2, 0,
                pre_kt={3: fin,
                        6: lambda: emit_qk(ktr, wk_sb, 1, 0, "k1_0"),
                        9: lambda: emit_qk(ktr, wk_sb, 1, 1, "k1_1"),
                        12: lambda: emit_qk(ktr, wk_sb, 1, 2, "k1_2")},
            )][1]
            fin = [fin, emit_attn(
                3, 0,
                pre_kt={3: fin,
                        6: lambda: emit_qk(ktr, wk_sb, 1, 3, "k1_3"),
                        9: lambda: emit_qk(qtr, wq_sb, 1, 0, "q1_0"),
                        12: lambda: emit_qk(qtr, wq_sb, 1, 1, "q1_1")},
            )][1]
            fin = [fin, emit_attn(
                0, 1,
                pre_kt={3: fin,
                        6: lambda: emit_qk(qtr, wq_sb, 1, 2, "q1_2"),
                        11: lambda: emit_qk(qtr, wq_sb, 1, 3, "q1_3")},
            )][1]
            fin = [fin, emit_attn(
                1, 1,
                pre_kt={3: fin, 6: lambda: emit_wo(0)},
            )][1]
            fin = [fin, emit_attn(
                2, 1,
                pre_kt={3: fin, 6: lambda: emit_wo(1)},
            )][1]
            fin = [fin, emit_attn(
                3, 1,
                pre_kt={3: fin, 6: lambda: emit_wo(2)},
            )][1]
            fin()
            emit_wo(3)

    _split_sync_waits(nc, maxw=1)
    return nc


_NC_CACHE = {}


def _rope_cos_sin(seq):
    inv_freq = 1.0 / (
        ROPE_BASE ** (np.arange(0, HD, 2, dtype=np.float32) / HD)
    )
    pos = np.arange(seq, dtype=np.float32)
    freqs = pos[:, None] * inv_freq[None, :]  # [seq, 32]
    emb = np.concatenate([freqs, freqs], axis=-1)  # [seq, 64]
    return np.cos(emb).astype(np.float32), np.sin(emb).astype(np.float32)


def build_in_maps(hs, W_qkv, b_qkv, W_o, b_o):
    """Per-core input dict list (shared by kernel() and test harness)."""
    seq = hs.shape[1]
    cos, sin = _rope_cos_sin(seq)
    # sign-folded sin: rows 0..31 negated (multiplies the shifted-up half)
    sin_hat = sin.copy()
    sin_hat[:, :32] *= -1.0
    cosT = np.ascontiguousarray(cos.T).astype(NPBF16)
    sinT = np.ascontiguousarray(sin_hat.T).astype(NPBF16)

    bq, bk = b_qkv[:D], b_qkv[D : 2 * D]
    in_maps = []
    for core in range(N_CORES):
        bb, g = core // 4, core % 4
        cols = slice(g * DLOC, (g + 1) * DLOC)
        in_maps.append(
            {
                "xT": np.ascontiguousarray(hs[bb].T).astype(NPBF16),
                "wq": np.ascontiguousarray(W_qkv[:, :D][:, cols]).astype(NPBF16),
                "wk": np.ascontiguousarray(
                    W_qkv[:, D : 2 * D][:, cols]
                ).astype(NPBF16),
                "wv": np.ascontiguousarray(W_qkv[:, 2 * D :][:, cols]).astype(
                    NPBF16
                ),
                "wo": np.ascontiguousarray(W_o[cols, :]).astype(NPBF16),
                "cosT": cosT,
                "sinT": sinT,
                "bqk": np.stack([bq[cols], bk[cols]]).astype(np.float32),
            }
        )
    return in_maps


def kernel(hidden_states, W_qkv, b_qkv, W_o, b_o):
    hs = np.asarray(hidden_states, dtype=np.float32)
    W_qkv = np.asarray(W_qkv, dtype=np.float32)
    b_qkv = np.asarray(b_qkv, dtype=np.float32)
    W_o = np.asarray(W_o, dtype=np.float32)
    b_o = np.asarray(b_o, dtype=np.float32)
    b, seq, d = hs.shape

    bq, bk, bv = b_qkv[:D], b_qkv[D : 2 * D], b_qkv[2 * D :]
    add_qk_bias = bool(np.any(bq) or np.any(bk))

    key = (seq, add_qk_bias)
    if key not in _NC_CACHE:
        _NC_CACHE[key] = build_attention_nc(seq, add_qk_bias)
    nc = _NC_CACHE[key]

    in_maps = build_in_maps(hs, W_qkv, b_qkv, W_o, b_o)
    res = run_bass_kernel_spmd(nc, in_maps, list(range(N_CORES)))
    parts = [res.results[c]["out"].astype(np.float32) for c in range(N_CORES)]
    outv = np.stack(
        [parts[0] + parts[1] + parts[2] + parts[3],
         parts[4] + parts[5] + parts[6] + parts[7]]
    )
    outv += b_o[None, None, :] + (bv @ W_o)[None, None, :]
    return outv.astype(np.float32)


# revision 14
# speedup vs baseline: 1.0935x; 1.0935x over previous
"""Trainium2 Bass kernel for CustomAttentionWithPE (bf16 pipeline).

Reference computation (B=2, S=2048, H=16, Dh=64, D=1024):
    qkv = hs @ W_qkv + b_qkv ; split to q,k,v per head
    q,k = RoPE(q), RoPE(k)
    out = softmax(q k^T / 8) v   (no mask)
    return concat_heads(out) @ W_o + b_o

Sharding: 8 cores -> (batch b = core//4, head-quad g = core%4, heads 4g..4g+3).
Each core computes partial = attn(heads of g, batch b) @ W_o[rows of g]
for its batch; host sums the 4 partials per batch and adds the bias terms
(b_o and b_v @ W_o; softmax rows sum to 1 so the V bias contributes exactly
b_v @ W_o per token).

v2 design (vs fp32 baseline at ~800us):
  - All matmuls in bf16 (1 cyc/row vs fp32's 4); inputs converted to bf16
    on host, halving DMA. PSUM accumulation stays fp32.
  - Softmax exp on ScalarE is the new bottleneck (~147us: 16.8M elems +
    352cyc/instr overhead). Program order starts attention as early as
    possible (K-hp0 + Q-qt0 projections first) and interleaves V/remaining
    QKV/WO under the exp-bound attention phases so ACT is never starved.
  - Score PSUM tiles are [128, 2(heads), 512] so ONE activation instruction
    exps both heads of a pair per k-tile (fewer fixed overheads).
  - 1/Z via reciprocal_approx_fast (f32, ~5x faster than DVE reciprocal)
    directly off the PSUM Z row; broadcast across partitions by a rank-1
    bf16 matmul into the pv bank's upper 64 partitions.
  - RoPE via host-precomputed sign-folded sin (rows 0-31 negated) so the
    rotate-half is 4 shifted tensor_tensor muls + mul + add, all bf16.
"""

import math
from contextlib import ExitStack

import numpy as np
import ml_dtypes

import concourse.bass as bass
import concourse.mybir as mybir
import concourse.tile as tile
from concourse.bass_utils import run_bass_kernel_spmd

F32 = mybir.dt.float32
BF16 = mybir.dt.bfloat16
AF = mybir.ActivationFunctionType
NPBF16 = ml_dtypes.bfloat16

B, S, D = 2, 2048, 1024
NH, HD = 16, 64
ROPE_BASE = 10000.0
N_CORES = 8
HPC = 4  # heads per core
DLOC = HPC * HD  # 256 local head dims per core


def _split_sync_waits(nc, maxw=1):
    """This container's walrus rejects >1-2 SyncWaits per instruction
    ("Too many sync wait commands"). Move excess waits onto NoOps."""
    for f in nc.m.functions:
        for blk in f.blocks:
            new_instructions = []
            for ins in blk.instructions:
                si = getattr(ins, "sync_info", None)
                if si is not None and si.on_wait and len(si.on_wait) > maxw:
                    waits = list(si.on_wait)
                    extra, keep = waits[:-maxw], waits[-maxw:]
                    si.on_wait = keep
                    for i in range(0, len(extra), maxw):
                        nop = mybir.InstNoOp(
                            name=nc.get_next_instruction_name(),
                            engine=ins.engine,
                            sync_info=mybir.SyncInfo(
                                on_wait=extra[i : i + maxw], on_update=[]
                            ),
                        )
                        nc.register_instruction(nop, overwrite=True)
                        new_instructions.append(nop)
                new_instructions.append(ins)
            blk.instructions[:] = new_instructions


def build_attention_nc(seq=S, add_qk_bias=False, order="interleaved"):
    """One SPMD program; per-core data differs only through inputs."""
    nc = bass.Bass()
    NT = seq // 512  # 512-token stripes
    KT = seq // 128  # k tiles
    NCH = D // 128  # contraction chunks over d_model

    xT = nc.dram_tensor("xT", [D, seq], BF16, kind="ExternalInput")
    wq = nc.dram_tensor("wq", [D, DLOC], BF16, kind="ExternalInput")
    wk = nc.dram_tensor("wk", [D, DLOC], BF16, kind="ExternalInput")
    wv = nc.dram_tensor("wv", [D, DLOC], BF16, kind="ExternalInput")
    wo = nc.dram_tensor("wo", [DLOC, D], BF16, kind="ExternalInput")
    cosT = nc.dram_tensor("cosT", [HD, seq], BF16, kind="ExternalInput")
    sinT = nc.dram_tensor("sinT", [HD, seq], BF16, kind="ExternalInput")
    bqk = nc.dram_tensor("bqk", [2, DLOC], F32, kind="ExternalInput")
    out = nc.dram_tensor("out", [seq, D], BF16, kind="ExternalOutput")

    with tile.TileContext(nc) as tc, ExitStack() as ctx:
        consts = ctx.enter_context(tc.tile_pool(name="consts", bufs=1))
        # warm the exp table set before any real activation
        warm = consts.tile([1, 2], F32)
        nc.vector.memset(warm, 0.0)
        nc.scalar.activation(warm, warm, AF.Exp, scale=1.0)

        # DMA dispatch order = need order: wk + rope tables + x stripe 0
        # feed the very first K projection; wq next (Q stripe 0); the rest
        # stream in behind them.
        wk_sb = consts.tile([128, NCH, DLOC], BF16)
        nc.sync.dma_start(out=wk_sb, in_=wk.rearrange("(c p) m -> p c m", p=128))
        cs_sb = consts.tile([128, seq], BF16)
        nc.sync.dma_start(out=cs_sb[0:HD, :], in_=cosT[:])
        nc.sync.dma_start(out=cs_sb[HD:128, :], in_=cosT[:])
        sn_sb = consts.tile([128, seq], BF16)
        nc.sync.dma_start(out=sn_sb[0:HD, :], in_=sinT[:])
        nc.sync.dma_start(out=sn_sb[HD:128, :], in_=sinT[:])
        x_sb = consts.tile([128, NCH, seq], BF16)

        def emit_x_stripe(nt):
            # one strided DMA moves the whole 512-token stripe (all 8 chunks)
            nc.sync.dma_start(
                out=x_sb[:, :, nt * 512 : nt * 512 + 512],
                in_=xT[:, nt * 512 : nt * 512 + 512].rearrange(
                    "(c p) m -> p c m", p=128
                ),
            )

        emit_x_stripe(0)
        wq_sb = consts.tile([128, NCH, DLOC], BF16)
        nc.sync.dma_start(out=wq_sb, in_=wq.rearrange("(c p) m -> p c m", p=128))
        for nt in range(1, NT):
            emit_x_stripe(nt)
        wv_sb = consts.tile([128, NCH, DLOC], BF16)
        nc.sync.dma_start(out=wv_sb, in_=wv.rearrange("(c p) m -> p c m", p=128))
        wo_sb = consts.tile([128, 2, D], BF16)
        nc.sync.dma_start(out=wo_sb, in_=wo.rearrange("(c p) m -> p c m", p=128))
        ones_sb = consts.tile([128, HD], BF16)
        nc.vector.memset(ones_sb, 1.0)
        if add_qk_bias:
            bqk_sb = consts.tile([128, 2, 2], F32)
            nc.sync.dma_start(
                out=bqk_sb, in_=bqk.rearrange("b (h p) -> p b h", p=128)
            )

        # long-lived activation tensors
        acts = ctx.enter_context(tc.tile_pool(name="acts", bufs=1))
        qtr = acts.tile([128, 2, seq], BF16)  # RoPE'd Q^T, head pairs
        ktr = acts.tile([128, 2, seq], BF16)
        v_sb = acts.tile([128, KT, HPC, HD + 1], BF16)  # V natural + ones col
        att = acts.tile([128, 2, seq], BF16)  # normalized attn out ^T
        nc.vector.memset(v_sb[:, :, :, HD : HD + 1], 1.0)

        # pools
        ps = ctx.enter_context(tc.tile_pool(name="ps", bufs=1, space="PSUM"))
        rope_pool = ctx.enter_context(tc.tile_pool(name="rope", bufs=3))
        pt_pool = ctx.enter_context(tc.tile_pool(name="ptp", bufs=6))
        npool = ctx.enter_context(tc.tile_pool(name="norm", bufs=2))
        opool = ctx.enter_context(tc.tile_pool(name="ostage", bufs=2))

        def emit_qk(dst, w_sb, hp, nt, name):
            """Project one [128 dims, 512 tok] stripe of q^T or k^T and RoPE it."""
            cs = slice(nt * 512, nt * 512 + 512)
            pst = ps.tile([128, 512], F32, tag="qkv", bufs=2, name=f"ps_{name}")
            for c in range(NCH):
                nc.tensor.matmul(
                    pst,
                    w_sb[:, c, hp * 128 : hp * 128 + 128],
                    x_sb[:, c, cs],
                    start=(c == 0),
                    stop=(c == NCH - 1),
                )
            raw = rope_pool.tile([128, 512], BF16, tag="raw", name=f"raw_{name}")
            nc.vector.tensor_copy(raw, pst)
            if add_qk_bias:
                bi = 0 if dst is qtr else 1
                nc.vector.tensor_scalar_add(raw, raw, bqk_sb[:, bi, hp : hp + 1])
            rot = rope_pool.tile([128, 512], BF16, tag="rot", name=f"rot_{name}")
            for b in (0, 64):
                nc.vector.tensor_copy(rot[b : b + 32, :], raw[b + 32 : b + 64, :])
                nc.vector.tensor_copy(rot[b + 32 : b + 64, :], raw[b : b + 32, :])
            nc.vector.tensor_mul(rot, rot, sn_sb[:, cs])
            tmp = rope_pool.tile([128, 512], BF16, tag="tmp", name=f"tmp_{name}")
            nc.vector.tensor_mul(tmp, raw, cs_sb[:, cs])
            nc.vector.tensor_add(dst[:, hp, cs], tmp, rot)

        def emit_v_chunk(kt_idx):
            """V natural [128 tokens, vcol] for one k-tile, all 4 heads."""
            vp = ps.tile([128, HPC, HD], F32, tag="qkv", bufs=2, name="ps_v")
            for c in range(NCH):
                nc.tensor.matmul(
                    vp,
                    x_sb[:, c, kt_idx * 128 : kt_idx * 128 + 128],
                    wv_sb[:, c, :],
                    start=(c == 0),
                    stop=(c == NCH - 1),
                )
            nc.vector.tensor_copy(v_sb[:, kt_idx, :, 0:HD], vp)

        def emit_v(nt):
            for tt in range(4):
                emit_v_chunk(nt * 4 + tt)

        def emit_attn(qt, hp, pre_kt=None, pre_pv=None):
            """scores -> exp -> PV -> normalize(A) for one (q-stripe, pair).

            Tile trace order IS program order, so hooks place prerequisite /
            filler work at exact trace positions:
              pre_kt[kt]: before that kt's score matmuls
              pre_pv[kt]: between the exp and that kt's PV matmuls (used for
                          the V chunk feeding exactly that PV)
            Returns a finish closure (norm part B: 1/Z broadcast + multiply)
            that the caller emits later — off the PE-critical path, after the
            reciprocal has had time to complete."""
            qs = slice(qt * 512, qt * 512 + 512)
            pv = [
                ps.tile([128, 512], F32, tag="pv", bufs=2, name="pv0"),
                ps.tile([128, 512], F32, tag="pv", bufs=2, name="pv1"),
            ]
            for kt in range(KT):
                if pre_kt and kt in pre_kt:
                    pre_kt[kt]()
                sc = ps.tile([128, 2, 512], F32, tag="sc", bufs=2, name="sc")
                for h in range(2):
                    hb = h * 64
                    nc.tensor.matmul(
                        sc[:, h, :],
                        ktr[hb : hb + 64, hp, kt * 128 : kt * 128 + 128],
                        qtr[hb : hb + 64, hp, qs],
                        start=True,
                        stop=True,
                    )
                pt = pt_pool.tile([128, 2, 512], BF16, tag="pt", bufs=6, name="pt")
                nc.scalar.activation(pt, sc, AF.Exp, scale=0.125)
                if pre_pv and kt in pre_pv:
                    pre_pv[kt]()
                for h in range(2):
                    nc.tensor.matmul(
                        pv[h][0 : HD + 1, :],
                        v_sb[:, kt, hp * 2 + h, :],
                        pt[:, h, :],
                        start=(kt == 0),
                        stop=(kt == KT - 1),
                        skip_group_check=True,
                    )
            # norm part A: stage O'/Z off PSUM (frees the pv slots) and start
            # the reciprocal. tensor_tensor needs same start partitions, so
            # partition shifts are done with copies (exempt from the rule).
            o_sb = npool.tile([128, 512], BF16, tag="osb", name="osb")
            zstage = npool.tile([65, 512], F32, tag="zstage", name="zstage")
            zrecf = npool.tile([65, 512], F32, tag="zrecf", name="zrecf")
            zrec = npool.tile([65, 512], BF16, tag="zrec", name="zrec")
            for h in range(2):
                hb = h * 64
                nc.vector.tensor_copy(o_sb[hb : hb + 64, :], pv[h][0:HD, :])
                nc.vector.tensor_copy(zstage[hb : hb + 1, :], pv[h][HD : HD + 1, :])
            # one partition-parallel reciprocal covers both heads' Z rows
            # (rows 1..63 are don't-care lanes; same wall time as [1,512])
            nc.vector.reciprocal(zrecf[0:HD + 1, :], zstage[0:HD + 1, :])
            for h in range(2):
                hb = h * 64
                nc.vector.tensor_copy(zrec[hb : hb + 1, :], zrecf[hb : hb + 1, :])

            def finish():
                # norm part B: rank-1 broadcast of 1/Z + un-normalized O' mul
                zb = ps.tile([128, 512], F32, tag="qkv", bufs=2, name="zb")
                for h in range(2):
                    hb = h * 64
                    nc.tensor.matmul(
                        zb[hb : hb + 64, :],
                        ones_sb[hb : hb + 1, 0:HD],
                        zrec[hb : hb + 1, :],
                        start=True,
                        stop=True,
                        skip_group_check=True,
                    )
                    nc.vector.tensor_mul(
                        att[hb : hb + 64, hp, qs],
                        o_sb[hb : hb + 64, :],
                        zb[hb : hb + 64, :],
                    )

            return finish

        def emit_wo(qt):
            """Output projection + store for one 512-token stripe."""
            for tt in range(4):
                tok = qt * 512 + tt * 128
                for nh in range(2):
                    pw = ps.tile([128, 512], F32, tag="qkv", bufs=2, name="ps_wo")
                    for hp in range(2):
                        nc.tensor.matmul(
                            pw,
                            att[:, hp, tok : tok + 128],
                            wo_sb[:, hp, nh * 512 : nh * 512 + 512],
                            start=(hp == 0),
                            stop=(hp == 1),
                        )
                    o_out = opool.tile([128, 512], BF16, tag="oo", name="oo")
                    nc.vector.tensor_copy(o_out, pw)
                    nc.sync.dma_start(
                        out=out[tok : tok + 128, nh * 512 : nh * 512 + 512],
                        in_=o_out,
                    )

        if order == "serial":
            for hp in range(2):
                for nt in range(NT):
                    emit_qk(ktr, wk_sb, hp, nt, f"k{hp}_{nt}")
                    emit_qk(qtr, wq_sb, hp, nt, f"q{hp}_{nt}")
            for nt in range(NT):
                emit_v(nt)
            for hp in range(2):
                for qt in range(NT):
                    emit_attn(qt, hp)()
            for qt in range(NT):
                emit_wo(qt)
        else:
            # Trace order = program order = scheduler priority. Attention
            # feeds the bottleneck ScalarE exp stream, so it leads; K/Q
            # stripe projections, V chunks, deferred norm finishes and WO are
            # threaded into exact positions where their outputs are first
            # needed (or as PE filler).
            emit_qk(ktr, wk_sb, 0, 0, "k0_0")
            emit_qk(qtr, wq_sb, 0, 0, "q0_0")
            fin = emit_attn(
                0, 0,
                pre_kt={1: lambda: emit_qk(ktr, wk_sb, 0, 1, "k0_1"),
                        4: lambda: emit_qk(ktr, wk_sb, 0, 2, "k0_2"),
                        8: lambda: emit_qk(ktr, wk_sb, 0, 3, "k0_3"),
                        12: lambda: emit_qk(qtr, wq_sb, 0, 1, "q0_1")},
                pre_pv={kt: (lambda k=kt: emit_v_chunk(k)) for kt in range(KT)},
            )
            fin = [fin, emit_attn(
                1, 0,
                pre_kt={2: lambda: emit_qk(qtr, wq_sb, 0, 2, "q0_2"),
                        8: fin,
                        12: lambda: emit_qk(qtr, wq_sb, 0, 3, "q0_3")},
            )][1]
            fin = [fin, emit_attn(
                2, 0,
                pre_kt={2: lambda: emit_qk(ktr, wk_sb, 1, 0, "k1_0"),
                        5: lambda: emit_qk(ktr, wk_sb, 1, 1, "k1_1"),
                        8: fin,
                        12: lambda: emit_qk(ktr, wk_sb, 1, 2, "k1_2")},
            )][1]
            fin = [fin, emit_attn(
                3, 0,
                pre_kt={2: lambda: emit_qk(ktr, wk_sb, 1, 3, "k1_3"),
                        5: lambda: emit_qk(qtr, wq_sb, 1, 0, "q1_0"),
                        8: fin,
                        12: lambda: emit_qk(qtr, wq_sb, 1, 1, "q1_1")},
            )][1]
            fin = [fin, emit_attn(
                0, 1,
                pre_kt={2: lambda: emit_qk(qtr, wq_sb, 1, 2, "q1_2"),
                        8: fin,
                        12: lambda: emit_qk(qtr, wq_sb, 1, 3, "q1_3")},
            )][1]
            fin = [fin, emit_attn(
                1, 1,
                pre_kt={8: fin, 11: lambda: emit_wo(0)},
            )][1]
            fin = [fin, emit_attn(
                2, 1,
                pre_kt={8: fin, 11: lambda: emit_wo(1)},
            )][1]
            fin = [fin, emit_attn(
                3, 1,
                pre_kt={8: fin, 11: lambda: emit_wo(2)},
            )][1]
            fin()
            emit_wo(3)

    _split_sync_waits(nc, maxw=1)
    return nc


_NC_CACHE = {}


def _rope_cos_sin(seq):
    inv_freq = 1.0 / (
        ROPE_BASE ** (np.arange(0, HD, 2, dtype=np.float32) / HD)
    )
    pos = np.arange(seq, dtype=np.float32)
    freqs = pos[:, None] * inv_freq[None, :]  # [seq, 32]
    emb = np.concatenate([freqs, freqs], axis=-1)  # [seq, 64]
    return np.cos(emb).astype(np.float32), np.sin(emb).astype(np.float32)


def build_in_maps(hs, W_qkv, b_qkv, W_o, b_o):
    """Per-core input dict list (shared by kernel() and test harness)."""
    seq = hs.shape[1]
    cos, sin = _rope_cos_sin(seq)
    # sign-folded sin: rows 0..31 negated (multiplies the shifted-up half)
    sin_hat = sin.copy()
    sin_hat[:, :32] *= -1.0
    cosT = np.ascontiguousarray(cos.T).astype(NPBF16)
    sinT = np.ascontiguousarray(sin_hat.T).astype(NPBF16)

    bq, bk = b_qkv[:D], b_qkv[D : 2 * D]
    in_maps = []
    for core in range(N_CORES):
        bb, g = core // 4, core % 4
        cols = slice(g * DLOC, (g + 1) * DLOC)
        in_maps.append(
            {
                "xT": np.ascontiguousarray(hs[bb].T).astype(NPBF16),
                "wq": np.ascontiguousarray(W_qkv[:, :D][:, cols]).astype(NPBF16),
                "wk": np.ascontiguousarray(
                    W_qkv[:, D : 2 * D][:, cols]
                ).astype(NPBF16),
                "wv": np.ascontiguousarray(W_qkv[:, 2 * D :][:, cols]).astype(
                    NPBF16
                ),
                "wo": np.ascontiguousarray(W_o[cols, :]).astype(NPBF16),
                "cosT": cosT,
                "sinT": sinT,
                "bqk": np.stack([bq[cols], bk[cols]]).astype(np.float32),
            }
        )
    return in_maps


def kernel(hidden_states, W_qkv, b_qkv, W_o, b_o):
    hs = np.asarray(hidden_states, dtype=np.float32)
    W_qkv = np.asarray(W_qkv, dtype=np.float32)
    b_qkv = np.asarray(b_qkv, dtype=np.float32)
    W_o = np.asarray(W_o, dtype=np.float32)
    b_o = np.asarray(b_o, dtype=np.float32)
    b, seq, d = hs.shape

    bq, bk, bv = b_qkv[:D], b_qkv[D : 2 * D], b_qkv[2 * D :]
    add_qk_bias = bool(np.any(bq) or np.any(bk))

    key = (seq, add_qk_bias)
    if key not in _NC_CACHE:
        _NC_CACHE[key] = build_attention_nc(seq, add_qk_bias)
    nc = _NC_CACHE[key]

    in_maps = build_in_maps(hs, W_qkv, b_qkv, W_o, b_o)
    res = run_bass_kernel_spmd(nc, in_maps, list(range(N_CORES)))
    parts = [res.results[c]["out"].astype(np.float32) for c in range(N_CORES)]
    outv = np.stack(
        [parts[0] + parts[1] + parts[2] + parts[3],
         parts[4] + parts[5] + parts[6] + parts[7]]
    )
    outv += b_o[None, None, :] + (bv @ W_o)[None, None, :]
    return outv.astype(np.float32)
